# revision 1
# baseline (speedup 1.0000x reference)
"""Trainium2 Bass kernel for the hyperbolic (Poincare-ball) GRU cell.

Data-parallel over batch across 8 NeuronCores, no collectives.

Key restructuring (validated against the jax reference in fp32 to ~7e-7):
  - zero_log(x) @ W = s_x * (x @ W): the log-map's per-row diagonal scaling
    commutes with the GEMM, so all six GEMMs run on RAW (host-pretransposed)
    activations; scales land on the GEMM outputs.
  - mobius_add(alpha_a*va, alpha_b*vb) = ua*va + ub*vb where ua/ub are per-row
    scalars built from row norms and the row dot <va, vb>.  All Poincare maps
    therefore reduce to per-row scalar chains + a few full-tensor passes.
  - Matmul operands bf16 (fp32 PSUM accumulate); everything else fp32.
"""

import threading
from contextlib import ExitStack

import ml_dtypes
import numpy as np

import concourse.bacc as bacc
import concourse.mybir as mybir
import concourse.tile as tile
from concourse.bass_utils import run_bass_kernel_spmd
from concourse.masks import make_identity

F32 = mybir.dt.float32
BF16 = mybir.dt.bfloat16
AF = mybir.ActivationFunctionType
OP = mybir.AluOpType
AX = mybir.AxisListType

N_CORES = 8
B, D = 4096, 2048
BL = B // N_CORES          # rows per core (512)
P = 128                    # partitions
NB = BL // P               # 4 batch tiles per core
KC = D // P                # 16 contraction chunks
JB = 512                   # GEMM j-block / PSUM bank width in fp32
NJ = D // JB               # 4 j-blocks

EPS = 1e-5
MAXN = 1.0 - 1e-5


def _build():
    nc = bacc.Bacc(None, target_bir_lowering=False, debug=False)

    x_d = nc.dram_tensor("x", [BL, D], F32, kind="ExternalInput")
    hx_d = nc.dram_tensor("hx", [BL, D], F32, kind="ExternalInput")
    xT_d = nc.dram_tensor("xT", [D, BL], BF16, kind="ExternalInput")
    hxT_d = nc.dram_tensor("hxT", [D, BL], BF16, kind="ExternalInput")
    w_d = {
        name: nc.dram_tensor(name, [D, D], BF16, kind="ExternalInput")
        for name in ["wTr", "uTr", "wTz", "uTz", "uTw", "wTw"]
    }
    b_d = {
        name: nc.dram_tensor(name, [P, D], F32, kind="ExternalInput")
        for name in ["br", "bz", "bw"]
    }
    out_d = nc.dram_tensor("out", [BL, D], F32, kind="ExternalOutput")

    with ExitStack() as ctx:
        tc = ctx.enter_context(tile.TileContext(nc))
        perm = ctx.enter_context(tc.tile_pool(name="perm", bufs=1))
        scal = ctx.enter_context(tc.tile_pool(name="scal", bufs=96))
        act = ctx.enter_context(tc.tile_pool(name="act", bufs=12))
        pmm = ctx.enter_context(tc.tile_pool(name="pmm", bufs=3, space="PSUM"))
        pscr = ctx.enter_context(tc.tile_pool(name="pscr", bufs=3, space="PSUM"))
        ptr = ctx.enter_context(tc.tile_pool(name="ptr", bufs=2, space="PSUM"))
        dram = ctx.enter_context(tc.tile_pool(name="dram", bufs=1, space="DRAM"))

        dve, sca, pe = nc.vector, nc.scalar, nc.tensor

        # ---------- helpers: per-row scalar tiles are [P, NB] (col = b-tile) --
        def stile(name="s"):
            return scal.tile([P, NB], F32, tag="scal", name=name)

        one_s = scal.tile([P, 1], F32, tag="one", name="one")
        dve = nc.vector
        dve.memset(one_s, 1.0)

        def sq_norms(v_tiles):
            """Row sum-of-squares over a full [BL, D] tensor -> [P, NB] tile.
            ACT Square pass per 512-block with accum_out, junk out to PSUM."""
            n2 = stile("n2")
            for bt in range(NB):
                part = scal.tile([P, NJ], F32, tag="part", name="part")
                for blk in range(NJ):
                    scr = pscr.tile([P, JB], F32, tag="scr", name="scr")
                    sca.activation(
                        out=scr,
                        in_=v_tiles[bt][:, blk * JB:(blk + 1) * JB],
                        func=AF.Square,
                        accum_out=part[:, blk:blk + 1],
                    )
                dve.tensor_reduce(n2[:, bt:bt + 1], part, AX.X, OP.add)
            return n2

        def row_dot(a_tiles, b_tiles):
            """Row dot of two full tensors -> [P, NB] tile.
            (tensor_tensor_reduce is broken on this terminal; use
            scalar_tensor_tensor's accum_out: out = (a*1)*b, accum = sum)"""
            dt_ = stile("dot")
            for bt in range(NB):
                part = scal.tile([P, NJ], F32, tag="part", name="part")
                for blk in range(NJ):
                    scr = pscr.tile([P, JB], F32, tag="scr", name="scr")
                    dve.scalar_tensor_tensor(
                        scr,
                        a_tiles[bt][:, blk * JB:(blk + 1) * JB],
                        one_s,
                        b_tiles[bt][:, blk * JB:(blk + 1) * JB],
                        OP.mult,
                        OP.mult,
                        accum_out=part[:, blk:blk + 1],
                    )
                dve.tensor_reduce(dt_[:, bt:bt + 1], part, AX.X, OP.add)
            return dt_

        def clip_unit(n):
            o = stile("nclip")
            dve.tensor_scalar(o, n, float(EPS), float(MAXN), OP.max, OP.min)
            return o

        def recip(n):
            o = stile("rec")
            dve.reciprocal(o, n)
            return o

        def artanh_over_n(ncl):
            """artanh(n)/n for clipped n: 0.5*ln((1+n)/(1-n))/n."""
            onep = stile("onep")
            dve.tensor_scalar(onep, ncl, 1.0, None, OP.add)
            onem = stile("onem")
            dve.tensor_scalar(onem, ncl, -1.0, 1.0, OP.mult, OP.add)
            u = stile("u")
            dve.tensor_tensor(u, onep, recip(onem), OP.mult)
            lnu = stile("lnu")
            sca.activation(lnu, u, AF.Ln)
            o = stile("aon")
            dve.scalar_tensor_tensor(o, lnu, 0.5, recip(ncl), OP.mult, OP.mult)
            return o

        def exp_scalars(n2):
            """(alpha, th): zero_exp(v) = alpha*v, ||zero_exp(v)|| = th."""
            n = stile("n")
            sca.activation(n, n2, AF.Sqrt)
            nm = stile("nm")
            dve.tensor_scalar(nm, n, float(EPS), None, OP.max)
            th = stile("th")
            sca.activation(th, nm, AF.Tanh)
            al = stile("al")
            dve.tensor_tensor(al, th, recip(nm), OP.mult)
            return al, th

        def log_scalars(n2):
            """beta: zero_log(m) = beta*m."""
            n = stile("n")
            sca.activation(n, n2, AF.Sqrt)
            return artanh_over_n(clip_unit(n))

        def mobius_scalars(al_a, th_a, al_b, th_b, dab, neg_a=False):
            """mobius_add(al_a*va, al_b*vb) = ua*va + ub*vb  ([P,NB] scalars).
            th_* may be tanh-norms (exp-map args) or raw-norm stand-ins; the
            caller passes x2/y2 via th^2.  neg_a negates the first argument."""
            x2 = stile("x2")
            dve.tensor_tensor(x2, th_a, th_a, OP.mult)
            y2 = stile("y2")
            dve.tensor_tensor(y2, th_b, th_b, OP.mult)
            xy = stile("xy")
            dve.tensor_tensor(xy, al_a, al_b, OP.mult)
            dve.tensor_tensor(xy, xy, dab, OP.mult)
            if neg_a:
                dve.tensor_scalar(xy, xy, -1.0, None, OP.mult)
            txy1 = stile("txy1")
            dve.tensor_scalar(txy1, xy, 2.0, 1.0, OP.mult, OP.add)
            numa = stile("numa")
            dve.tensor_tensor(numa, txy1, y2, OP.add)
            x2y2 = stile("x2y2")
            dve.tensor_tensor(x2y2, x2, y2, OP.mult)
            den = stile("den")
            dve.tensor_tensor(den, txy1, x2y2, OP.add)
            dve.tensor_scalar(den, den, float(EPS), None, OP.max)
            rden = recip(den)
            ua = stile("ua")
            dve.tensor_tensor(ua, numa, al_a, OP.mult)
            dve.tensor_tensor(ua, ua, rden, OP.mult)
            if neg_a:
                dve.tensor_scalar(ua, ua, -1.0, None, OP.mult)
            omx2 = stile("omx2")
            dve.tensor_scalar(omx2, x2, -1.0, 1.0, OP.mult, OP.add)
            ub = stile("ub")
            dve.tensor_tensor(ub, omx2, al_b, OP.mult)
            dve.tensor_tensor(ub, ub, rden, OP.mult)
            return ua, ub

        def combine(dst_tiles, a_tiles, ua, b_tiles, ub, out_pool=None):
            """dst = ua*a + ub*b per b-tile.  t = ub*b is written in place on b
            unless dst is a fresh tile (out_pool given)."""
            outs = []
            for bt in range(NB):
                if out_pool is None:
                    t = b_tiles[bt]
                    sca.activation(t, b_tiles[bt], AF.Copy, scale=ub[:, bt:bt + 1])
                    dve.scalar_tensor_tensor(
                        dst_tiles[bt], a_tiles[bt], ua[:, bt:bt + 1], t,
                        OP.mult, OP.add,
                    )
                    outs.append(dst_tiles[bt])
                else:
                    t = out_pool.tile([P, D], F32, tag="big", name="comb")
                    sca.activation(t, b_tiles[bt], AF.Copy, scale=ub[:, bt:bt + 1])
                    dve.scalar_tensor_tensor(
                        t, a_tiles[bt], ua[:, bt:bt + 1], t, OP.mult, OP.add
                    )
                    outs.append(t)
            return outs

        # ---------- stage 0: naturals + log-map scalars ----------
        def load_nat(src, pool):
            tiles = []
            for bt in range(NB):
                t = pool.tile([P, D], F32, tag="big", name="nat")
                nc.sync.dma_start(out=t, in_=src[bt * P:(bt + 1) * P, :])
                tiles.append(t)
            return tiles

        x_tiles = load_nat(x_d, act)
        n2x = sq_norms(x_tiles)
        x_tiles = None
        hx_tiles = load_nat(hx_d, act)
        n2h = sq_norms(hx_tiles)    # raw Sum(hx^2): mobius x2 term, kept to end
        hx_tiles = None             # reloaded later

        s_x = log_scalars(n2x)
        s_h = log_scalars(n2h)

        # biases (host pre-broadcast to [P, D])
        bias_sb = {}
        for name in ["br", "bz", "bw"]:
            t = perm.tile([P, D], F32, tag=f"bias_{name}", name=name)
            nc.sync.dma_start(out=t, in_=b_d[name][:, :])
            bias_sb[name] = t

        ident = perm.tile([P, P], BF16, tag="ident", name="ident")
        make_identity(nc, ident)

        wslab_pool = {}

        def load_T(src):
            t = actT_pool["p"].tile([P, KC, BL], BF16, tag="aT", name="aT")
            nc.sync.dma_start(
                out=t, in_=src[:, :].rearrange("(c p) b -> p c b", p=P)
            )
            return t

        def gemm_phase(wt_dram, lhsT, scale, bias_tile):
            """v = scale*(act @ w^T) (+bias) streamed by j-slab.
            Returns (v_tiles, n2) with v in act pool."""
            v_tiles = [
                act.tile([P, D], F32, tag="big", name="v") for _ in range(NB)
            ]
            for js in range(NJ):
                slab = wslab_pool["p"].tile(
                    [P, KC, JB], BF16, tag="wslab", name="wslab"
                )
                nc.sync.dma_start(
                    out=slab,
                    in_=wt_dram[:, js * JB:(js + 1) * JB].rearrange(
                        "(c p) j -> p c j", p=P
                    ),
                )
                for bt in range(NB):
                    ps = pmm.tile([P, JB], F32, tag="mm", name="mm")
                    for c in range(KC):
                        pe.matmul(
                            ps,
                            lhsT[:, c, bt * P:(bt + 1) * P],
                            slab[:, c, :],
                            start=(c == 0),
                            stop=(c == KC - 1),
                        )
                    dst = v_tiles[bt][:, js * JB:(js + 1) * JB]
                    if bias_tile is not None:
                        dve.scalar_tensor_tensor(
                            dst, ps, scale[:, bt:bt + 1],
                            bias_tile[:, js * JB:(js + 1) * JB],
                            OP.mult, OP.add,
                        )
                    else:
                        dve.tensor_scalar_mul(dst, ps, scale[:, bt:bt + 1])
            return v_tiles, sq_norms(v_tiles)

        actT_pool = {}
        with tc.tile_pool(name="actT", bufs=2) as _actT:
            actT_pool["p"] = _actT
            with tc.tile_pool(name="wpool", bufs=2) as _wp:
                wslab_pool["p"] = _wp

                hxT_sb = load_T(hxT_d)
                v1, n2_1 = gemm_phase(w_d["wTr"], hxT_sb, s_h, None)
                v3, n2_3 = gemm_phase(w_d["wTz"], hxT_sb, s_h, None)
                hxT_sb = None
                xT_sb = load_T(xT_d)
                v2, n2_2 = gemm_phase(w_d["uTr"], xT_sb, s_x, bias_sb["br"])

                # ----- r = sigmoid(beta * mobius(E(v1), E(v2))) -----
                al1, th1 = exp_scalars(n2_1)
                al2, th2 = exp_scalars(n2_2)
                ua, ub = mobius_scalars(al1, th1, al2, th2, row_dot(v1, v2))
                m1 = combine(v1, v1, ua, v2, ub)       # m1 in v1 slots
                b1 = log_scalars(sq_norms(m1))
                r_tiles = v2                            # sigmoid into v2 slots
                for bt in range(NB):
                    sca.activation(
                        r_tiles[bt], m1[bt], AF.Sigmoid, scale=b1[:, bt:bt + 1]
                    )
                v1 = m1 = None

                v4, n2_4 = gemm_phase(w_d["uTz"], xT_sb, s_x, bias_sb["bz"])

                # ----- z -----
                al3, th3 = exp_scalars(n2_3)
                al4, th4 = exp_scalars(n2_4)
                ua, ub = mobius_scalars(al3, th3, al4, th4, row_dot(v3, v4))
                m2 = combine(v3, v3, ua, v4, ub)
                b2 = log_scalars(sq_norms(m2))
                z_tiles = v4
                for bt in range(NB):
                    sca.activation(
                        z_tiles[bt], m2[bt], AF.Sigmoid, scale=b2[:, bt:bt + 1]
                    )
                v3 = m2 = None

                # spill z; reloaded in the tail
                z_spill = dram.tile([BL, D], F32, tag="zsp", name="zsp")
                for bt in range(NB):
                    nc.sync.dma_start(
                        out=z_spill[bt * P:(bt + 1) * P, :], in_=z_tiles[bt]
                    )
                z_tiles = None

                v5, n2_5 = gemm_phase(w_d["uTw"], xT_sb, s_x, bias_sb["bw"])
                xT_sb = None

                # ----- p = r*hx (bf16), PE-transpose into pT -----
                hx_tiles = load_nat(hx_d, act)
                pT_sb = actT_pool["p"].tile([P, KC, BL], BF16, tag="aT", name="pT")
                for bt in range(NB):
                    for cp in range(KC // 4):
                        pbf = act.tile([P, JB], BF16, tag="pbf", bufs=3, name="pbf")
                        dve.tensor_tensor(
                            pbf,
                            r_tiles[bt][:, cp * JB:(cp + 1) * JB],
                            hx_tiles[bt][:, cp * JB:(cp + 1) * JB],
                            OP.mult,
                        )
                        ps = ptr.tile([P, JB], BF16, tag="tr", name="tr")
                        for k in range(4):
                            pe.transpose(
                                ps[:, k * P:(k + 1) * P],
                                pbf[:, k * P:(k + 1) * P],
                                ident,
                            )
                        dve.tensor_copy(
                            out=pT_sb[:, cp * 4:cp * 4 + 4, bt * P:(bt + 1) * P],
                            in_=ps.rearrange("p (c b) -> p c b", c=4),
                        )
                r_tiles = None

                v6, n2_6 = gemm_phase(w_d["wTw"], pT_sb, s_h, None)
                pT_sb = None

        # ----- tail: m3, q, inter, d, e, out (weight pools closed) -----
        with tc.tile_pool(name="tailp", bufs=8) as tailp:
            ones_t = stile("ones")
            dve.memset(ones_t, 1.0)
            al6, th6 = exp_scalars(n2_6)
            al5, th5 = exp_scalars(n2_5)
            ua, ub = mobius_scalars(al6, th6, al5, th5, row_dot(v6, v5))
            m3 = combine(v6, v6, ua, v5, ub)            # m3 in v6 slots
            b3 = log_scalars(sq_norms(m3))
            q_tiles = v5                                 # tanh into v5 slots
            for bt in range(NB):
                sca.activation(q_tiles[bt], m3[bt], AF.Tanh, scale=b3[:, bt:bt + 1])
            v5 = v6 = m3 = None

            # inter = zero_exp(q): delta = tanh(nq)/nq, ||inter|| = thq
            n2q = sq_norms(q_tiles)
            deltas = exp_scalars(n2q)
            delta, thq = deltas

            # d = mobius(-hx, delta*q) = ua*hx + ub*q  (x2 from raw |hx|^2)
            sqrt_n2h = stile("nh_raw")
            sca.activation(sqrt_n2h, n2h, AF.Sqrt)
            ua, ub = mobius_scalars(
                # al_a = 1 (hx enters raw), th_a = sqrt(n2h) so x2 = n2h
                ones_t, sqrt_n2h, delta, thq, row_dot(hx_tiles, q_tiles),
                neg_a=True,
            )
            d_tiles = combine(None, hx_tiles, ua, q_tiles, ub, out_pool=tailp)
            q_tiles = None

            # L(d): beta_d; e = z*d; t2 = E(beta_d * e)
            beta_d = log_scalars(sq_norms(d_tiles))

            z_tiles = []
            for bt in range(NB):
                t = tailp.tile([P, D], F32, tag="big", name="zre")
                nc.sync.dma_start(out=t, in_=z_spill[bt * P:(bt + 1) * P, :])
                z_tiles.append(t)
            e_tiles = d_tiles
            for bt in range(NB):
                dve.tensor_tensor(e_tiles[bt], z_tiles[bt], d_tiles[bt], OP.mult)
            z_tiles = None

            n2e = sq_norms(e_tiles)
            ne = stile("ne")
            sca.activation(ne, n2e, AF.Sqrt)
            nt = stile("nt")
            dve.tensor_tensor(nt, beta_d, ne, OP.mult)
            dve.tensor_scalar(nt, nt, float(EPS), None, OP.max)
            tht = stile("tht")
            sca.activation(tht, nt, AF.Tanh)
            eps_s = stile("eps_s")
            dve.tensor_tensor(eps_s, tht, recip(nt), OP.mult)
            dve.tensor_tensor(eps_s, eps_s, beta_d, OP.mult)

            # out = mobius(hx, eps*e) = ua*hx + ub*e
            ua, ub = mobius_scalars(
                ones_t, sqrt_n2h, eps_s, tht, row_dot(hx_tiles, e_tiles)
            )
            outs = combine(e_tiles, hx_tiles, ua, e_tiles, ub)
            for bt in range(NB):
                nc.sync.dma_start(out=out_d[bt * P:(bt + 1) * P, :], in_=outs[bt])

    nc.compile()
    return nc


def _build_null():
    """Same I/O signature, DMA-only body — used to calibrate dispatch+transfer
    overhead when measuring the real kernel's device time."""
    nc = bacc.Bacc(None, target_bir_lowering=False, debug=False)
    nc.dram_tensor("x", [BL, D], F32, kind="ExternalInput")
    hx_d = nc.dram_tensor("hx", [BL, D], F32, kind="ExternalInput")
    nc.dram_tensor("xT", [D, BL], BF16, kind="ExternalInput")
    nc.dram_tensor("hxT", [D, BL], BF16, kind="ExternalInput")
    for name in ["wTr", "uTr", "wTz", "uTz", "uTw", "wTw"]:
        nc.dram_tensor(name, [D, D], BF16, kind="ExternalInput")
    for name in ["br", "bz", "bw"]:
        nc.dram_tensor(name, [P, D], F32, kind="ExternalInput")
    out_d = nc.dram_tensor("out", [BL, D], F32, kind="ExternalOutput")
    with ExitStack() as ctx:
        tc = ctx.enter_context(tile.TileContext(nc))
        pool = ctx.enter_context(tc.tile_pool(name="p", bufs=2))
        for bt in range(NB):
            t = pool.tile([P, D], F32, tag="t", name="t")
            nc.sync.dma_start(out=t, in_=hx_d[bt * P:(bt + 1) * P, :])
            nc.sync.dma_start(out=out_d[bt * P:(bt + 1) * P, :], in_=t)
    nc.compile()
    return nc


_BUILD_LOCK = threading.Lock()
_NC_CACHE = {}


def _get_nc():
    with _BUILD_LOCK:
        if "nc" not in _NC_CACHE:
            _NC_CACHE["nc"] = _build()
        return _NC_CACHE["nc"]


def kernel(**inputs: np.ndarray) -> np.ndarray:
    x = np.ascontiguousarray(np.asarray(inputs["x"], dtype=np.float32))
    hx = np.ascontiguousarray(np.asarray(inputs["hx"], dtype=np.float32))
    bf = ml_dtypes.bfloat16

    def wT(a):
        return np.ascontiguousarray(np.asarray(a, dtype=np.float32).T).astype(bf)

    weights = {
        "wTr": wT(inputs["w_r"]),
        "uTr": wT(inputs["u_r_w"]),
        "wTz": wT(inputs["w_z"]),
        "uTz": wT(inputs["u_z_w"]),
        "uTw": wT(inputs["u_w"]),
        "wTw": wT(inputs["w"]),
    }
    biases = {
        "br": np.ascontiguousarray(
            np.broadcast_to(np.asarray(inputs["u_r_b"], np.float32), (P, D))
        ),
        "bz": np.ascontiguousarray(
            np.broadcast_to(np.asarray(inputs["u_z_b"], np.float32), (P, D))
        ),
        "bw": np.ascontiguousarray(
            np.broadcast_to(np.asarray(inputs["u_b"], np.float32), (P, D))
        ),
    }

    in_maps = []
    for c in range(N_CORES):
        xs = x[c * BL:(c + 1) * BL]
        hs = hx[c * BL:(c + 1) * BL]
        m = {
            "x": xs,
            "hx": hs,
            "xT": np.ascontiguousarray(xs.T).astype(bf),
            "hxT": np.ascontiguousarray(hs.T).astype(bf),
        }
        m.update(weights)
        m.update(biases)
        in_maps.append(m)

    nc = _get_nc()
    res = run_bass_kernel_spmd(nc, in_maps, core_ids=list(range(N_CORES)))
    return np.concatenate([r["out"] for r in res.results], axis=0)

